# revision 1
# baseline (speedup 1.0000x reference)
"""AttentionWithLoRA on 8 Trainium2 NeuronCores.

Three SPMD phases (identical program per core, different data):
  K1  row-sharded (512 tokens/core): LayerNorm -> transpose -> QKV projection
      (+ LoRA), emitting qT/kT [INNER, 512] and v [512, INNER] layouts.
  K2  (batch, head-group)-sharded (4 heads/core): block-causal attention.
      scores are computed transposed [k, q]; exp without max-subtraction
      (logits are O(10) here); V is augmented with a ones column so the
      softmax denominator falls out of the same accumulating matmul.
      Output is the *unnormalized* attn @ v_aug, [65 rows/head, T].
  K3  row-sharded: normalize by the denominator, output projection + LoRA,
      emitted transposed [DIM, 512] and transposed back on the host.

All big matmuls run as float32r (full PE rate at N>=256, ~1e-4 rel err).
LayerNorm gamma/beta and all LoRA scale factors are folded on the host.
"""

import functools
import os

import numpy as np

import concourse.bacc as bacc
import concourse.mybir as mybir
import concourse.tile as tile
from concourse.bass_utils import run_bass_kernel_spmd
from concourse.masks import make_identity

F32 = mybir.dt.float32
F32R = mybir.dt.float32r
AF = mybir.ActivationFunctionType
ALU = mybir.AluOpType

B = 2
T = 2048
DIM = 1024
H = 16
DH = 64
INNER = 1024
RANK = 16
LORA_SCALE = 0.5
NF = 8            # frames
NP = 256          # patches per frame
EPS = 1e-5
NCORES = 8
TOK = (B * T) // NCORES       # 512 tokens per core in K1/K3
HPC = H // (NCORES // B)      # 4 heads per core in K2
SCALE = DH ** -0.5            # 0.125

# collected exec times (ns) when KERNEL_TRACE=1
TRACE_NS = []


def _trace_on():
    return bool(int(os.environ.get("KERNEL_TRACE", "0")))


PHASE_WALL = []
_RUNNERS = {}


def _get_runner(nc):
    """Jitted SPMD executor for a finalized Bass module, cached across calls
    (run_bass_kernel_spmd re-traces jax.jit on every invocation)."""
    key = id(nc)
    if key in _RUNNERS:
        return _RUNNERS[key]
    import jax
    import jax.numpy as jnp  # noqa: F401
    from jax.experimental.shard_map import shard_map
    from jax.sharding import Mesh, PartitionSpec
    import concourse.mybir as mybir_
    from concourse import bass2jax

    bass2jax.install_neuronx_cc_hook()
    partition_name = (
        nc.partition_id_tensor.name if nc.partition_id_tensor else None)
    in_names, out_names, out_avals, zero_outs = [], [], [], []
    for alloc in nc.m.functions[0].allocations:
        if not isinstance(alloc, mybir_.MemoryLocationSet):
            continue
        name = alloc.memorylocations[0].name
        if alloc.kind == "ExternalInput":
            if name != partition_name:
                in_names.append(name)
        elif alloc.kind == "ExternalOutput":
            shape = tuple(alloc.tensor_shape)
            dtype = mybir_.dt.np(alloc.dtype)
            out_names.append(name)
            out_avals.append(jax.core.ShapedArray(shape, dtype))
            zero_outs.append(np.zeros(shape, dtype))
    n_params = len(in_names)
    n_outs = len(out_avals)
    all_in = list(in_names) + list(out_names)
    if partition_name is not None:
        all_in.append(partition_name)
    donate = tuple(range(n_params, n_params + n_outs))

    def _body(*args):
        operands = list(args)
        if partition_name is not None:
            operands.append(bass2jax.partition_id_tensor())
        outs = bass2jax._bass_exec_p.bind(
            *operands,
            out_avals=tuple(out_avals),
            in_names=tuple(all_in),
            out_names=tuple(out_names),
            lowering_input_output_aliases=(),
            sim_require_finite=True,
            sim_require_nnan=True,
            nc=nc,
        )
        return tuple(outs)

    devices = jax.devices()[:NCORES]
    mesh = Mesh(np.asarray(devices), ("core",))
    in_specs = (PartitionSpec("core"),) * (n_params + n_outs)
    out_specs = (PartitionSpec("core"),) * n_outs
    # no donation: every output element is written by the kernels, so the
    # zero buffers are only placeholder operands -- upload them once instead
    # of shipping fresh zeros over the axon tunnel on every call
    sharded = jax.jit(
        shard_map(_body, mesh=mesh, in_specs=in_specs, out_specs=out_specs,
                  check_rep=False),
        keep_unused=True,
    )
    from jax.sharding import NamedSharding
    shd = NamedSharding(mesh, PartitionSpec("core"))
    dev_zeros = [
        jax.device_put(
            np.zeros((NCORES * z.shape[0], *z.shape[1:]), z.dtype), shd)
        for z in zero_outs
    ]
    input_cache = {}

    def run(in_maps):
        import hashlib
        concat_in = []
        for name in in_names:
            arrs = [np.asarray(m[name]) for m in in_maps]
            replicated = all(a is arrs[0] for a in arrs)
            if replicated:
                digest = hashlib.md5(arrs[0].tobytes()).hexdigest()
                hit = input_cache.get(name)
                if hit is not None and hit[0] == digest:
                    concat_in.append(hit[1])
                    continue
                dev = jax.device_put(
                    np.concatenate(arrs, axis=0), shd)
                input_cache[name] = (digest, dev)
                concat_in.append(dev)
            else:
                concat_in.append(np.concatenate(arrs, axis=0))
        out_arrs = sharded(*concat_in, *dev_zeros)
        return [
            {
                name: np.asarray(out_arrs[i])
                .reshape(NCORES, *out_avals[i].shape)[c]
                for i, name in enumerate(out_names)
            }
            for c in range(NCORES)
        ]

    _RUNNERS[key] = run
    return run


def _run(nc, in_maps):
    import time as _time
    t0 = _time.time()
    if _trace_on():
        r = run_bass_kernel_spmd(
            nc, in_maps, core_ids=list(range(NCORES)), trace=True,
        )
        if r.exec_time_ns:
            TRACE_NS.append(r.exec_time_ns)
        PHASE_WALL.append(_time.time() - t0)
        return r.results
    results = _get_runner(nc)(in_maps)
    PHASE_WALL.append(_time.time() - t0)
    return results


# --------------------------------------------------------------------------
# K1: LayerNorm + QKV + LoRA.  512 tokens/core, full width.
# --------------------------------------------------------------------------
@functools.lru_cache(maxsize=None)
def _build_k1():
    nc = bacc.Bacc()
    x_i = nc.dram_tensor("x_rows", [TOK, DIM], F32, kind="ExternalInput")
    wqk_i = nc.dram_tensor("w_qk", [128, 16 * 8 * 128], F32, kind="ExternalInput")
    wv_i = nc.dram_tensor("w_v", [128, 8 * INNER], F32, kind="ExternalInput")
    cc_i = nc.dram_tensor("c_col", [128, 16], F32, kind="ExternalInput")
    cv_i = nc.dram_tensor("c_vrow", [128, INNER], F32, kind="ExternalInput")
    qT_o = nc.dram_tensor("qT_o", [INNER, TOK], F32, kind="ExternalOutput")
    kT_o = nc.dram_tensor("kT_o", [INNER, TOK], F32, kind="ExternalOutput")
    v_o = nc.dram_tensor("v_o", [TOK, INNER], F32, kind="ExternalOutput")

    NT = TOK // 128   # 4 token tiles
    ND = DIM // 128   # 8 dim chunks

    with tile.TileContext(nc) as tc:
        with (
            tc.tile_pool(name="const", bufs=1) as const,
            tc.tile_pool(name="wblk", bufs=3) as wblk,
            tc.tile_pool(name="xio", bufs=4) as xio,
            tc.tile_pool(name="stats", bufs=2) as stats,
            tc.tile_pool(name="evac", bufs=3) as evac,
            tc.tile_pool(name="ps_tr", bufs=2, space="PSUM") as ps_tr,
            tc.tile_pool(name="ps_qk", bufs=3, space="PSUM") as ps_qk,
        ):
            # token tiles first on the HW queue: LN is the critical path
            x_ts = []
            for t in range(NT):
                x_t = xio.tile([128, DIM], F32, name="x_t", tag="x")
                nc.sync.dma_start(out=x_t, in_=x_i[t * 128:(t + 1) * 128, :])
                x_ts.append(x_t)

            # constants on the SWDGE queue, smallest/gating first
            cc_sb = const.tile([128, 16], F32, name="cc_sb")
            nc.gpsimd.dma_start(out=cc_sb, in_=cc_i[:, :])
            cv_sb = const.tile([128, INNER], F32, name="cv_sb")
            nc.gpsimd.dma_start(out=cv_sb, in_=cv_i[:, :])
            ident = const.tile([128, 128], F32, name="ident")
            make_identity(nc, ident)
            eps_t = const.tile([128, 1], F32, name="eps_t")
            nc.vector.memset(eps_t, EPS)

            xnT_sb = const.tile([128, ND, TOK], F32R, name="xnT_sb")

            # LayerNorm + transpose, one 128-token tile at a time
            for t in range(NT):
                x_t = x_ts[t]
                st = stats.tile([128, 2, 6], F32, name="st")
                for i in range(2):
                    nc.vector.bn_stats(out=st[:, i, :], in_=x_t[:, i * 512:(i + 1) * 512])
                mv = stats.tile([128, 2], F32, name="mv")
                nc.vector.bn_aggr(out=mv, in_=st)
                sd = stats.tile([128, 1], F32, name="sd")
                nc.scalar.activation(out=sd, in_=mv[:, 1:2], func=AF.Sqrt,
                                     bias=eps_t, scale=1.0)
                rstd = stats.tile([128, 1], F32, name="rstd")
                nc.vector.reciprocal(out=rstd, in_=sd)
                nmr = stats.tile([128, 1], F32, name="nmr")
                nc.vector.scalar_tensor_tensor(
                    out=nmr, in0=mv[:, 0:1], scalar=-1.0, in1=rstd,
                    op0=ALU.mult, op1=ALU.mult,
                )
                xn_t = xio.tile([128, DIM], F32, name="xn_t", tag="xn")
                nc.scalar.activation(
                    out=xn_t, in_=x_t, func=AF.Identity, bias=nmr, scale=rstd,
                )
                for g in range(2):
                    tr_ps = ps_tr.tile([128, 4, 128], F32, name="tr_ps")
                    for j in range(4):
                        do = g * 4 + j
                        nc.tensor.transpose(
                            tr_ps[:, j, :], xn_t[:, do * 128:(do + 1) * 128], ident)
                    dstv = xnT_sb[:, g * 4:(g + 1) * 4, t * 128:(t + 1) * 128]
                    if g == 0:
                        nc.vector.tensor_copy(out=dstv, in_=tr_ps)
                    else:
                        nc.scalar.copy(out=dstv, in_=tr_ps)

            # qT / kT: 16 column blocks of 128, streamed weights, paired
            # output DMAs; v (tb, ch) units interleaved into the stream
            wv_sb = const.tile([128, 2, ND, 512], F32R, name="wv_sb")
            v2_tiles = {}

            def emit_qk_pair(cbp):
                qk2 = evac.tile([128, 2, TOK], F32, name="qk2", tag="qk2")
                for half in range(2):
                    cb = cbp * 2 + half
                    wt = wblk.tile([128, 8 * 128], F32R, name="wt", tag="wqk")
                    nc.sync.dma_start(
                        out=wt, in_=wqk_i[:, cb * 1024:(cb + 1) * 1024].bitcast(F32R),
                    )
                    qk_ps = ps_qk.tile([128, TOK], F32, name="qk_ps", tag="acc")
                    for do in range(ND):
                        nc.tensor.matmul(
                            qk_ps, wt[:, do * 128:(do + 1) * 128],
                            xnT_sb[:, do, :],
                            start=(do == 0), stop=(do == ND - 1),
                        )
                    nc.vector.tensor_scalar_add(
                        out=qk2[:, half, :], in0=qk_ps, scalar1=cc_sb[:, cb:cb + 1],
                    )
                dst = qT_o if cbp < 4 else kT_o
                r0 = (cbp % 4) * 256
                nc.sync.dma_start(
                    out=dst[r0:r0 + 256, :].rearrange("(c p) t -> p c t", p=128),
                    in_=qk2,
                )

            def emit_wv(ch, h):
                nc.sync.dma_start(
                    out=wv_sb[:, ch, h * 4:(h + 1) * 4, :],
                    in_=wv_i[:, (ch * 2 + h) * 2048:(ch * 2 + h + 1) * 2048]
                    .rearrange("p (c d) -> p c d", c=4).bitcast(F32R),
                )

            def emit_v_unit(tb, ch):
                if tb not in v2_tiles:
                    v2_tiles[tb] = evac.tile(
                        [128, INNER], F32, name=f"v2_{tb}", tag=f"v2_{tb}")
                v2 = v2_tiles[tb]
                v_ps = ps_qk.tile([128, 512], F32, name="v_ps", tag="acc")
                for do in range(ND):
                    nc.tensor.matmul(
                        v_ps, xnT_sb[:, do, tb * 128:(tb + 1) * 128],
                        wv_sb[:, ch, do, :],
                        start=(do == 0), stop=(do == ND - 1),
                    )
                nc.vector.tensor_tensor(
                    out=v2[:, ch * 512:(ch + 1) * 512], in0=v_ps,
                    in1=cv_sb[:, ch * 512:(ch + 1) * 512], op=ALU.add,
                )
                if ch == 1:
                    nc.sync.dma_start(
                        out=v_o[tb * 128:(tb + 1) * 128, :], in_=v2,
                    )

            schedule = [
                (0, [("wv", 0, 0)]),
                (1, [("wv", 0, 1)]),
                (2, [("v", 0, 0)]),
                (3, [("wv", 1, 0), ("v", 1, 0)]),
                (4, [("wv", 1, 1), ("v", 2, 0)]),
                (5, [("v", 3, 0), ("v", 0, 1)]),
                (6, [("v", 1, 1), ("v", 2, 1)]),
                (7, [("v", 3, 1)]),
            ]
            for cbp, extras in schedule:
                emit_qk_pair(cbp)
                for e in extras:
                    if e[0] == "wv":
                        emit_wv(e[1], e[2])
                    else:
                        emit_v_unit(e[1], e[2])

    nc.finalize()
    return nc


# --------------------------------------------------------------------------
# K2: block-causal attention for 4 heads of one batch.
# --------------------------------------------------------------------------
@functools.lru_cache(maxsize=None)
def _build_k2():
    nc = bacc.Bacc()
    qT_i = nc.dram_tensor("qT_h", [HPC * DH, T], F32, kind="ExternalInput")
    kT_i = nc.dram_tensor("kT_h", [HPC * DH, T], F32, kind="ExternalInput")
    va_i = nc.dram_tensor("vaug_h", [T, HPC * (DH + 1)], F32, kind="ExternalInput")
    ao_o = nc.dram_tensor("ao_o", [HPC * (DH + 1), T], F32, kind="ExternalOutput")

    NKC = T // 128   # 16 key chunks
    GRP = 4          # ST chunks per exp batch

    with tile.TileContext(nc) as tc:
        with (
            tc.tile_pool(name="inp", bufs=1) as inp,
            tc.tile_pool(name="expp", bufs=6) as expp,
            tc.tile_pool(name="aop", bufs=8) as aop,
            tc.tile_pool(name="ps_st", bufs=3, space="PSUM") as ps_st,
            tc.tile_pool(name="ps_av", bufs=2, space="PSUM") as ps_av,
        ):
            qT_sb = inp.tile([128, 2, T], F32R, name="qT_sb")
            kT_sb = inp.tile([128, 2, T], F32R, name="kT_sb")
            v_sb = inp.tile([128, NKC, HPC * (DH + 1)], F32R, name="v_sb")
            qT_r = qT_i.rearrange("(c p) t -> p c t", p=128).bitcast(F32R)
            kT_r = kT_i.rearrange("(c p) t -> p c t", p=128).bitcast(F32R)
            va_r = va_i.rearrange("(c p) n -> p c n", p=128).bitcast(F32R)
            for ch in range(2):
                for hf in range(4):
                    sl = slice(hf * (T // 4), (hf + 1) * (T // 4))
                    nc.sync.dma_start(out=kT_sb[:, ch, sl], in_=kT_r[:, ch, sl])
                    nc.sync.dma_start(out=qT_sb[:, ch, sl], in_=qT_r[:, ch, sl])
                    ksl = slice(ch * 8 + hf * 2, ch * 8 + (hf + 1) * 2)
                    nc.sync.dma_start(out=v_sb[:, ksl, :], in_=va_r[:, ksl, :])

            for h in range(HPC):
                base = 64 * (h % 2)
                ch = h // 2
                qs = qT_sb[base:base + DH, ch, :]
                ks = kT_sb[base:base + DH, ch, :]
                for f in range(NF):
                    nk = 2 * (f + 1)          # 128-key chunks this frame sees
                    av_ps = ps_av.tile([DH + 1, NP], F32, name="av_ps")
                    for g0 in range(0, nk, GRP):
                        gn = min(GRP, nk - g0)
                        st_ps = ps_st.tile([128, GRP * NP], F32, name="st_ps")
                        for j in range(gn):
                            kc = g0 + j
                            nc.tensor.matmul(
                                st_ps[:, j * NP:(j + 1) * NP],
                                ks[:, kc * 128:(kc + 1) * 128],
                                qs[:, f * NP:(f + 1) * NP],
                                start=True, stop=True,
                            )
                        st_exp = expp.tile([128, GRP * NP], F32R, name="st_exp")
                        nc.scalar.activation(
                            out=st_exp[:, :gn * NP], in_=st_ps[:, :gn * NP],
                            func=AF.Exp, scale=SCALE,
                        )
                        for j in range(gn):
                            kc = g0 + j
                            nc.tensor.matmul(
                                av_ps, v_sb[:, kc, h * (DH + 1):(h + 1) * (DH + 1)],
                                st_exp[:, j * NP:(j + 1) * NP],
                                start=(kc == 0), stop=(kc == nk - 1),
                            )
                    ao_sb = aop.tile([DH + 1, NP], F32, name="ao_sb")
                    nc.vector.tensor_copy(out=ao_sb, in_=av_ps)
                    r0 = h * (DH + 1)
                    nc.sync.dma_start(
                        out=ao_o[r0:r0 + DH + 1, f * NP:(f + 1) * NP], in_=ao_sb,
                    )

    nc.finalize()
    return nc


# --------------------------------------------------------------------------
# K3: normalize + output projection + LoRA, transposed output.
# --------------------------------------------------------------------------
@functools.lru_cache(maxsize=None)
def _build_k3():
    nc = bacc.Bacc()
    ao_i = nc.dram_tensor("aoT_c", [INNER, TOK], F32, kind="ExternalInput")
    di_i = nc.dram_tensor("dinv_p", [2, 8 * TOK], F32, kind="ExternalInput")
    se_i = nc.dram_tensor("dsel", [2, 128], F32, kind="ExternalInput")
    w_i = nc.dram_tensor("w_out", [INNER, DIM], F32, kind="ExternalInput")
    be_i = nc.dram_tensor("bobe_col", [128, 8], F32, kind="ExternalInput")
    o_o = nc.dram_tensor("outT_o", [DIM, TOK], F32, kind="ExternalOutput")

    NI = INNER // 128  # 8
    NDC = DIM // 128   # 8

    with tile.TileContext(nc) as tc:
        with (
            tc.tile_pool(name="inp", bufs=1) as inp,
            tc.tile_pool(name="evac", bufs=4) as evac,
            tc.tile_pool(name="ps_o", bufs=8, space="PSUM") as ps_o,
        ):
            # compact 1/denominator: [2, NI*TOK] pair-rows expanded on the PE
            # via a K=2 selector matmul (64x row replication without shipping
            # the expanded tensor over HBM)
            di_sb = inp.tile([2, NI, TOK], F32R, name="di_sb")
            nc.gpsimd.dma_start(
                out=di_sb, in_=di_i.rearrange("p (c t) -> p c t", c=NI).bitcast(F32R))
            se_sb = inp.tile([2, 128], F32R, name="se_sb")
            nc.gpsimd.dma_start(out=se_sb, in_=se_i[:, :].bitcast(F32R))
            rep_sb = inp.tile([128, NI, TOK], F32, name="rep_sb")
            for ic in range(NI):
                rep_ps = ps_o.tile([128, TOK], F32, name="rep_ps", tag="ops")
                nc.tensor.matmul(rep_ps, se_sb, di_sb[:, ic, :],
                                 start=True, stop=True)
                nc.vector.tensor_copy(out=rep_sb[:, ic, :], in_=rep_ps)

            ao_raw = inp.tile([128, NI, TOK], F32, name="ao_raw")
            w_sb = inp.tile([128, NI, DIM], F32R, name="w_sb")
            ao_n = inp.tile([128, NI, TOK], F32R, name="ao_n")
            ao_r = ao_i.rearrange("(c p) t -> p c t", p=128)
            w_r = w_i.rearrange("(c p) d -> p c d", p=128).bitcast(F32R)
            for ic in range(NI):
                nc.sync.dma_start(out=ao_raw[:, ic, :], in_=ao_r[:, ic, :])
                nc.sync.dma_start(out=w_sb[:, ic, :], in_=w_r[:, ic, :])
                nc.vector.tensor_tensor(
                    out=ao_n[:, ic, :], in0=ao_raw[:, ic, :], in1=rep_sb[:, ic, :],
                    op=ALU.mult,
                )

            be_sb = inp.tile([128, 8], F32, name="be_sb")
            nc.gpsimd.dma_start(out=be_sb, in_=be_i[:, :])

            # outT = w_out_eff^T @ attn_out^T (+ b_eff), single pass over 8
            # held banks; the out-LoRA is folded into w_out_eff on the host so
            # evacuations go straight to the output
            o_tiles = [
                ps_o.tile([128, TOK], F32, name=f"o_ps{dc}", tag="ops")
                for dc in range(NDC)
            ]
            for ic in range(NI):
                for dc in range(NDC):
                    nc.tensor.matmul(
                        o_tiles[dc], w_sb[:, ic, dc * 128:(dc + 1) * 128],
                        ao_n[:, ic, :],
                        start=(ic == 0), stop=(ic == NI - 1),
                    )
            for dcp in range(4):
                fin = evac.tile([128, 2, TOK], F32, name="fin")
                for half in range(2):
                    dc = dcp * 2 + half
                    if half == 0:
                        nc.vector.tensor_scalar_add(
                            out=fin[:, half, :], in0=o_tiles[dc],
                            scalar1=be_sb[:, dc:dc + 1],
                        )
                    else:
                        nc.scalar.activation(
                            out=fin[:, half, :], in_=o_tiles[dc], func=AF.Identity,
                            bias=be_sb[:, dc:dc + 1], scale=1.0,
                        )
                nc.sync.dma_start(
                    out=o_o[dcp * 256:(dcp + 1) * 256, :]
                    .rearrange("(c p) t -> p c t", p=128),
                    in_=fin,
                )

    nc.finalize()
    return nc


# --------------------------------------------------------------------------
# host orchestration
# --------------------------------------------------------------------------
def kernel(x, memory_tokens, ln_g, ln_b, w_qkv, qA, qB, kA, kB, vA, vB,
           w_out, b_out, outA, outB, bias_mask):
    f32 = np.float32
    x = np.ascontiguousarray(np.asarray(x, f32)).reshape(B * T, DIM)
    ln_g = np.asarray(ln_g, f32)
    ln_b = np.asarray(ln_b, f32)
    w_qkv = np.asarray(w_qkv, f32)
    w_out_n = np.ascontiguousarray(np.asarray(w_out, f32))
    b_out = np.asarray(b_out, f32)
    outA_n = np.ascontiguousarray(np.asarray(outA, f32))
    outB_n = np.asarray(outB, f32)

    # fold the LoRA deltas into the projection weights (exact:
    # xn@W + s(xn@A)@B = xn@(W + s A@B)), then LN gamma into the rows;
    # beta contributes constant row vectors
    w_lora = np.asarray(w_qkv, f32).copy()
    for s, (Am, Bm) in enumerate([(qA, qB), (kA, kB), (vA, vB)]):
        w_lora[:, s * INNER:(s + 1) * INNER] += LORA_SCALE * (
            np.asarray(Am, f32) @ np.asarray(Bm, f32))
    w_eff = np.ascontiguousarray(ln_g[:, None] * w_lora)
    a_eff_full = None  # qkv LoRA fully folded
    c_all = (ln_b @ w_lora).astype(f32)             # [3*INNER]
    c_col = np.ascontiguousarray(
        c_all[:2 * INNER].reshape(16, 128).T).astype(f32)      # [128, 16]
    c_vrow = np.ascontiguousarray(
        np.broadcast_to(c_all[2 * INNER:], (128, INNER))).astype(f32)

    # ---- K1
    # w_qk blocked: [128 part, cb, do, 128] so each 128-col block is one DMA
    w_qk_r = np.ascontiguousarray(
        w_eff[:, :2 * INNER].reshape(8, 128, 16, 128)
        .transpose(1, 2, 0, 3).reshape(128, 16 * 8 * 128))
    w_v_r = np.ascontiguousarray(
        w_eff[:, 2 * INNER:].reshape(8, 128, 2, 512)
        .transpose(1, 2, 0, 3).reshape(128, 8 * INNER))
    nc1 = _build_k1()
    in1 = [
        dict(x_rows=x[c * TOK:(c + 1) * TOK], w_qk=w_qk_r, w_v=w_v_r,
             c_col=c_col, c_vrow=c_vrow)
        for c in range(NCORES)
    ]
    r1 = _run(nc1, in1)

    qT_b = [np.concatenate([r1[4 * b_ + j]["qT_o"] for j in range(4)], axis=1)
            for b_ in range(B)]
    kT_b = [np.concatenate([r1[4 * b_ + j]["kT_o"] for j in range(4)], axis=1)
            for b_ in range(B)]
    v_b = [np.concatenate([r1[4 * b_ + j]["v_o"] for j in range(4)], axis=0)
           for b_ in range(B)]

    # ---- K2
    nc2 = _build_k2()
    ones = np.ones((T, HPC, 1), f32)
    in2 = []
    for c in range(NCORES):
        b_, hg = divmod(c, NCORES // B)
        r0 = hg * HPC * DH
        vau = np.concatenate(
            [v_b[b_][:, r0:r0 + HPC * DH].reshape(T, HPC, DH), ones], axis=2,
        ).reshape(T, HPC * (DH + 1))
        in2.append(dict(
            qT_h=np.ascontiguousarray(qT_b[b_][r0:r0 + HPC * DH]),
            kT_h=np.ascontiguousarray(kT_b[b_][r0:r0 + HPC * DH]),
            vaug_h=np.ascontiguousarray(vau),
        ))
    r2 = _run(nc2, in2)

    # rows h*(DH+1)..h*(DH+1)+DH-1 = unnormalized attn-out, last row = denom
    aoT_b, dinv_b = [], []
    for b_ in range(B):
        full = np.concatenate(
            [r2[(NCORES // B) * b_ + hg]["ao_o"] for hg in range(NCORES // B)],
            axis=0,
        ).reshape(H, DH + 1, T)
        aoT_b.append(np.ascontiguousarray(full[:, :DH, :].reshape(INNER, T)))
        dinv_b.append(np.ascontiguousarray(1.0 / full[:, DH, :]))

    # ---- K3
    return _run_k3(aoT_b, dinv_b, w_out_n, b_out, outA_n, outB_n)


def _run_k3(aoT_b, dinv_b, w_out_n, b_out, outA_n, outB_n):
    f32 = np.float32
    nc3 = _build_k3()
    # fold the out-LoRA into the projection weights (exact: P + s*P@A@B
    # with P = ao@w_out + b  ==  ao@(w_out + s*(w_out@A)@B) + (b + s*(b@A)@B))
    w_out_eff = np.ascontiguousarray(
        w_out_n + (w_out_n @ outA_n) @ (LORA_SCALE * outB_n))
    b_eff = (b_out + LORA_SCALE * ((b_out @ outA_n) @ outB_n)).astype(f32)
    be_col = np.ascontiguousarray(b_eff.reshape(8, 128).T)
    dsel = np.zeros((2, 128), f32)
    dsel[0, :64] = 1.0
    dsel[1, 64:] = 1.0
    in3 = []
    for c in range(NCORES):
        b_, j = divmod(c, NCORES // B)
        sl = slice(j * TOK, (j + 1) * TOK)
        dip = np.ascontiguousarray(
            dinv_b[b_][:, sl].reshape(8, 2, TOK).transpose(1, 0, 2)
            .reshape(2, 8 * TOK))
        in3.append(dict(
            aoT_c=np.ascontiguousarray(aoT_b[b_][:, sl]), dinv_p=dip,
            dsel=dsel, w_out=w_out_eff, bobe_col=be_col,
        ))
    r3 = _run(nc3, in3)

    out = np.empty((B * T, DIM), f32)
    for c in range(NCORES):
        out[c * TOK:(c + 1) * TOK] = r3[c]["outT_o"].T
    return out.reshape(B, T, DIM)

